# revision 26
# baseline (speedup 1.0000x reference)
"""Trainium2 Bass kernel for a dense transformer layer (attention + FFN).

Sharding: 8 shards = (batch b, sequence half) pairs. Each core computes the
full K/V projections for its batch (2x redundant) and Q/attention/FFN for its
1024-token query slice. No cross-core communication.

On-device layout is feature-major (transposed): activations live as
[feature, token] so every matmul is lhsT.T @ rhs with natural weight layouts.

Precision: QKV/O projections and the attention ctx matmuls run fp8e4 with
DoubleRow perf mode (256-row contraction per instruction, ~1.9x PE speedup);
kt/qt/E/v/ctx live in fp8. The FFN stays bf16 end to end — fp8 there costs
~2.4e-2 relative error (measured), over the 2e-2 gate. Weights are pre-scaled
x16 on the host to sit in fp8e4's normal range; the 1/16 is folded into the
PSUM->SBUF readout ops. ctx is scaled x32 via the softmax reciprocal
(compensated in Wo). fp32 PSUM accumulation; residual stream in fp32.

Schedule: the query slice is split in two 512-token halves. Phase A runs
K/Q/V projections and half-0 attention (ACT-bound on exp). Phase B runs
half-1 attention on ACT while the PE stream interleaves half-0's O
projection and FFN between attention matmuls — keeping the PE array dense
(avoids HAM down-throttle) and overlapping the exp floor with FFN compute.
Phase C finishes half-1's O projection and FFN.
"""

import numpy as np
import ml_dtypes

B, S, D = 4, 2048, 1024
H, DH, F = 16, 64, 4096
P = 128
NCORES = 8
SQ = B * S // NCORES  # 1024 query tokens per core
HQ = SQ // 2  # 512-token query half
DC = D // P  # 8 feature chunks
DC2 = DC // 2  # 4 double chunks (DoubleRow)
FC = F // P  # 32 ffn chunks
SKC = S // P  # 16 key chunks
SKC2 = SKC // 2
NPAIR = H // 2  # 8 head pairs (2 heads per 128-feature chunk)

WS = 16.0  # host-side fp8 weight scale
RWS = 1.0 / WS
CTXS = 32.0  # ctx fp8 scale (folded into softmax recip; compensated in Wo)

# Partial-fp8 FFN: the first F1C (of DC2=4) 256-row double-chunks of the
# FFN1 contraction and the first F2C (of FC2=16) of FFN2 run fp8 DoubleRow;
# the rest stay bf16. (2, 8) measured 1.92e-2 on HW — too close to the 2e-2
# gate; (2, 4) sims at ~1.55e-2.
F1C = 2
F2C = 4
D8 = 2 * F1C  # y/W1 chunks stored fp8
F8 = 2 * F2C  # h/W2 chunks stored fp8
# f2a/f2b matmul split point so the two units are balanced
NB1 = max(0, (F2C + FC - F8) // 2 - F2C)

BF16 = ml_dtypes.bfloat16
FP8 = ml_dtypes.float8_e4m3

_CACHE = {}


def _build_program():
    import concourse.mybir as mybir
    import concourse.tile as tile
    from concourse import bacc

    f32 = mybir.dt.float32
    bf16 = mybir.dt.bfloat16
    fp8 = mybir.dt.float8e4
    AF = mybir.ActivationFunctionType
    DR = mybir.MatmulPerfMode.DoubleRow
    MUL = mybir.AluOpType.mult
    ADD = mybir.AluOpType.add

    nc = bacc.Bacc("TRN2", target_bir_lowering=False, debug=False, num_devices=NCORES)

    xT_d = nc.dram_tensor("xT", [P, DC, S], fp8, kind="ExternalInput")
    xqT_d = nc.dram_tensor("xqT", [P, DC, SQ], fp8, kind="ExternalInput")
    xres_d = nc.dram_tensor("xres", [P, DC, SQ], f32, kind="ExternalInput")
    wq_d = nc.dram_tensor("wq", [P, DC, D], fp8, kind="ExternalInput")
    wk_d = nc.dram_tensor("wk", [P, DC, D], fp8, kind="ExternalInput")
    wv_d = nc.dram_tensor("wv", [P, DC, D], fp8, kind="ExternalInput")
    wo_d = nc.dram_tensor("wo", [P, DC, D], fp8, kind="ExternalInput")
    w1a_d = nc.dram_tensor("w1a", [FC, P, D8, P], fp8, kind="ExternalInput")
    w1b_d = nc.dram_tensor("w1b", [FC, P, DC - D8, P], bf16, kind="ExternalInput")
    w2a_d = nc.dram_tensor("w2a", [DC, P, F8, P], fp8, kind="ExternalInput")
    w2b_d = nc.dram_tensor("w2b", [DC, P, FC - F8, P], bf16, kind="ExternalInput")
    bq_d = nc.dram_tensor("bq", [P, DC], f32, kind="ExternalInput")
    bk_d = nc.dram_tensor("bk", [P, DC], f32, kind="ExternalInput")
    bvb_d = nc.dram_tensor("bvb", [P, D], bf16, kind="ExternalInput")
    b1_d = nc.dram_tensor("b1", [P, FC], f32, kind="ExternalInput")
    b2_d = nc.dram_tensor("b2", [P, DC], f32, kind="ExternalInput")
    outT_d = nc.dram_tensor("outT", [P, DC, SQ], f32, kind="ExternalOutput")

    with tile.TileContext(nc) as tc:
        with (
            tc.tile_pool(name="psA", bufs=2, space="PSUM") as psA,
            tc.tile_pool(name="psS", bufs=2, space="PSUM") as psS,
            tc.tile_pool(name="psC", bufs=2, space="PSUM") as psC,
            tc.tile_pool(name="biasp", bufs=1) as biasp,
            tc.tile_pool(name="ctxp", bufs=1) as ctxp,
            tc.tile_pool(name="ep", bufs=6) as ep,
            tc.tile_pool(name="rp", bufs=2) as rp,
            tc.tile_pool(name="rbp", bufs=2) as rbp,
        ):
            bq_sb = biasp.tile([P, DC], f32)
            bk_sb = biasp.tile([P, DC], f32)
            b1_sb = biasp.tile([P, FC], f32)
            b2_sb = biasp.tile([P, DC], f32)
            nc.sync.dma_start(bq_sb[:], bq_d[:])
            nc.sync.dma_start(bk_sb[:], bk_d[:])
            nc.sync.dma_start(b1_sb[:], b1_d[:])
            nc.sync.dma_start(b2_sb[:], b2_d[:])

            ctxT_sb = ctxp.tile([P, DC, SQ], fp8)
            # y (FFN input activation), feature-chunk-split by FFN1 dtype
            acc8_sb = ctxp.tile([P, D8, SQ], fp8)
            acc16_sb = ctxp.tile([P, DC - D8, SQ], bf16)
            wo_sb = ctxp.tile([P, DC, D], fp8)
            v_sb = ctxp.tile([P, SKC, H, DH + 1], fp8)
            kt_all = ctxp.tile([P, NPAIR, S], fp8)
            qt_all = ctxp.tile([P, NPAIR, SQ], fp8)
            yT_sb = ctxp.tile([P, DC, SQ], f32)

            def attn_pair(p, half, spread_hook=None):
                """Attention for head pair (2p, 2p+1), queries
                [half*HQ, half*HQ+HQ). spread_hook(skh) emits filler PE work.

                Software-pipelined: scores run one sk-chunk-pair ahead of ctx
                so the in-order PE stream never serializes the next scores
                behind exp — ACT stays continuously busy on exp."""
                q0 = half * HQ
                pc0 = psC.tile([P, HQ], f32, tag="pc")
                pc1 = psC.tile([P, HQ], f32, tag="pc")
                E2s = [None] * SKC2

                def emit_scores(skh):
                    # E2: exp(scores), laid [key, chunk-parity, headA|headB]
                    # = the ctx DoubleRow moving operand.
                    E2 = ep.tile([P, 2, 2 * HQ], fp8)
                    E2s[skh] = E2
                    for hs in range(2):
                        sk = 2 * skh + hs
                        ss = psS.tile([P, 2 * HQ], f32)
                        nc.tensor.matmul(
                            ss[:, 0:HQ],
                            kt_all[0:64, p, sk * P : (sk + 1) * P],
                            qt_all[0:64, p, q0 : q0 + HQ],
                            start=True,
                            stop=True,
                        )
                        nc.tensor.matmul(
                            ss[:, HQ : 2 * HQ],
                            kt_all[64:128, p, sk * P : (sk + 1) * P],
                            qt_all[64:128, p, q0 : q0 + HQ],
                            start=True,
                            stop=True,
                        )
                        nc.scalar.activation(E2[:, hs, :], ss, AF.Exp)

                emit_scores(0)
                for skh in range(SKC2):
                    if skh + 1 < SKC2:
                        emit_scores(skh + 1)
                    # filler PE work lands between next-scores and this ctx so
                    # the PE covers the exp latency instead of stalling on E2
                    if spread_hook is not None:
                        spread_hook(skh)
                    E2 = E2s[skh]
                    nc.tensor.matmul(
                        pc0[:65],
                        v_sb[:, 2 * skh : 2 * skh + 2, 2 * p, :],
                        E2[:, :, 0:HQ],
                        start=(skh == 0),
                        stop=(skh == SKC2 - 1),
                        perf_mode=DR,
                    )
                    nc.tensor.matmul(
                        pc1[:65],
                        v_sb[:, 2 * skh : 2 * skh + 2, 2 * p + 1, :],
                        E2[:, :, HQ : 2 * HQ],
                        start=(skh == 0),
                        stop=(skh == SKC2 - 1),
                        perf_mode=DR,
                    )
                # softmax normalization: ctx * (CTXS / rowsum); the CTXS fp8
                # range scale is divided back out in Wo. (approx recip is ~18
                # correct bits, plenty for a softmax denom)
                for hh, pc in ((0, pc0), (1, pc1)):
                    s0 = rp.tile([1, HQ], f32, tag="s")
                    nc.vector.tensor_scalar_mul(s0, pc[64:65, :], 1.0 / CTXS)
                    r0 = rp.tile([1, HQ], f32, tag="r")
                    nc.vector.reciprocal_approx_fast(r0, s0)
                    rb0 = rbp.tile([64, HQ], f32, tag="rb")
                    nc.gpsimd.partition_broadcast(rb0, r0)
                    nc.vector.tensor_mul(
                        ctxT_sb[64 * hh : 64 * hh + 64, p, q0 : q0 + HQ],
                        pc[0:64, :],
                        rb0,
                    )

            # ---------------- Phase A: projections + half-0 attention -------
            with (
                tc.tile_pool(name="abp", bufs=1) as abp,
                tc.tile_pool(name="wvp", bufs=1) as wvp,
                tc.tile_pool(name="ws", bufs=3) as ws,
            ):
                # x^T in 4 column-chunk tiles so V/K matmuls start after the
                # first chunk lands rather than after the full DMA.
                xTs = [
                    abp.tile([P, DC, 512], fp8, tag=f"xT{c}", name=f"xT{c}")
                    for c in range(4)
                ]
                wvs = [
                    wvp.tile([P, DC, 512], fp8, tag=f"wv{c}", name=f"wv{c}")
                    for c in range(2)
                ]
                bvb_sb = abp.tile([P, D], bf16)
                xqT_sb = abp.tile([P, DC, SQ], fp8)
                # startup DMAs spread across engine queues: descriptor gen is
                # ~0.7us per dma_start, so the first V matmul's inputs (x and
                # wv chunks 0-1) go first as single descriptors
                nc.sync.dma_start(xTs[0][:, 0:2, :], xT_d[:, 0:2, 0:512])
                nc.gpsimd.dma_start(wvs[0][:, 0:2, :], wv_d[:, 0:2, 0:512])
                nc.sync.dma_start(xTs[0][:, 2:DC, :], xT_d[:, 2:DC, 0:512])
                nc.gpsimd.dma_start(wvs[0][:, 2:DC, :], wv_d[:, 2:DC, 0:512])
                nc.scalar.dma_start(xTs[1][:], xT_d[:, :, 512:1024])
                nc.sync.dma_start(xTs[2][:], xT_d[:, :, 1024:1536])
                nc.scalar.dma_start(xTs[3][:], xT_d[:, :, 1536:2048])
                nc.gpsimd.dma_start(wvs[1][:], wv_d[:, :, 512:1024])
                nc.gpsimd.dma_start(xqT_sb[:], xqT_d[:])
                nc.scalar.dma_start(wo_sb[:], wo_d[:])
                nc.sync.dma_start(bvb_sb[:], bvb_d[:])

                # V projection, token-major: v[sk, dv] (+ ones column per
                # head). fp8: it is the ctx DoubleRow stationary operand.
                nc.vector.memset(v_sb[:, :, :, DH : DH + 1], 1.0)

                def emit_v(nv, sks, h0=0, h1=8):
                    nh = h1 - h0
                    for sk in sks:
                        xt = xTs[sk // 4]
                        co = (sk % 4) * P
                        ps = psA.tile([P, 512], f32, tag="ps")
                        for c in range(DC2):
                            nc.tensor.matmul(
                                ps[:, : nh * DH],
                                xt[:, 2 * c : 2 * c + 2, co : co + P],
                                wvs[nv][:, 2 * c : 2 * c + 2, h0 * DH : h1 * DH],
                                start=(c == 0),
                                stop=(c == DC2 - 1),
                                perf_mode=DR,
                            )
                        nc.vector.scalar_tensor_tensor(
                            v_sb[:, sk, nv * 8 + h0 : nv * 8 + h1, 0:DH],
                            ps[:, : nh * DH].rearrange("p (h d) -> p h d", h=nh),
                            RWS,
                            bvb_sb[
                                :, nv * 512 + h0 * DH : nv * 512 + h1 * DH
                            ].rearrange("p (h d) -> p h d", h=nh),
                            MUL,
                            ADD,
                        )

                def kq_units(p):
                    """K/Q projection PE work for pair p as 6 ~1-1.5us units.
                    Weight DMAs are issued at queue-build time (prefetch)."""
                    wkt = ws.tile([P, DC, P], fp8, tag="wchunk")
                    nc.sync.dma_start(wkt[:], wk_d[:, :, p * P : (p + 1) * P])
                    wqt = ws.tile([P, DC, P], fp8, tag="wchunk")
                    nc.sync.dma_start(wqt[:], wq_d[:, :, p * P : (p + 1) * P])
                    units = []
                    for n in range(S // 512):
                        def ku(n=n):
                            ps = psA.tile([P, 512], f32, tag="ps")
                            for c in range(DC2):
                                nc.tensor.matmul(
                                    ps,
                                    wkt[:, 2 * c : 2 * c + 2, :],
                                    xTs[n][:, 2 * c : 2 * c + 2, :],
                                    start=(c == 0),
                                    stop=(c == DC2 - 1),
                                    perf_mode=DR,
                                )
                            nc.vector.tensor_scalar(
                                kt_all[:, p, n * 512 : (n + 1) * 512],
                                ps,
                                RWS,
                                bk_sb[:, p : p + 1],
                                MUL,
                                ADD,
                            )
                        units.append(ku)
                    # wq is x16 overall on the host (x128 on Wq*scale for fp8
                    # range); divide the full 128 back out in the readout.
                    for n in range(SQ // 512):
                        def qu(n=n):
                            ps = psA.tile([P, 512], f32, tag="ps")
                            for c in range(DC2):
                                nc.tensor.matmul(
                                    ps,
                                    wqt[:, 2 * c : 2 * c + 2, :],
                                    xqT_sb[:, 2 * c : 2 * c + 2, n * 512 : (n + 1) * 512],
                                    start=(c == 0),
                                    stop=(c == DC2 - 1),
                                    perf_mode=DR,
                                )
                            nc.vector.tensor_scalar(
                                qt_all[:, p, n * 512 : (n + 1) * 512],
                                ps,
                                RWS / 8.0,
                                bq_sb[:, p : p + 1],
                                MUL,
                                ADD,
                            )
                        units.append(qu)
                    return units

                # V(nv=1) chunk counts per pair (heads 8-15, needed from
                # pair 4 on — must complete by end of pair 3)
                V1_PLAN = {0: 2, 1: 5, 2: 5, 3: 4}

                emit_v(0, range(SKC))
                for u in kq_units(0):
                    u()
                v1_next = 0
                for p in range(NPAIR):
                    queue = []
                    if p + 1 < NPAIR:
                        queue.extend(kq_units(p + 1))
                    for _ in range(V1_PLAN.get(p, 0)):
                        queue.append(lambda sk=v1_next: emit_v(1, [sk]))
                        v1_next += 1

                    def hook(skh, queue=queue):
                        # drain >=1 unit/slot, catching up so the queue
                        # empties by the last slot of the pair
                        rem_slots = SKC2 - skh
                        n = max(1, -(-len(queue) // rem_slots))
                        for _ in range(min(n, len(queue))):
                            queue.pop(0)()

                    attn_pair(p, half=0, spread_hook=hook)
                    for u in queue:
                        u()

            # ------- Phases B+C: half-1 attention overlapped with half-0
            # O-projection + FFN, then half-1 O + FFN ---------------------
            with (
                tc.tile_pool(name="htp", bufs=1) as htp,
                tc.tile_pool(name="w1s", bufs=3) as w1s,
                tc.tile_pool(name="w2s", bufs=2) as w2s,
                tc.tile_pool(name="xrp", bufs=3) as xrp,
                tc.tile_pool(name="outp", bufs=4) as outp,
            ):
                def ffn_units(half):
                    """Yield per-unit closures of O-proj + FFN PE work for one
                    query half. Each unit is ~1-2us of PE work."""
                    q0 = half * HQ
                    # O projection + residual: one unit per feature chunk m
                    for m in range(DC):
                        def o_unit(m=m):
                            xr = xrp.tile([P, HQ], f32, tag="xr")
                            nc.sync.dma_start(xr[:], xres_d[:, m, q0 : q0 + HQ])
                            ps = psA.tile([P, HQ], f32)
                            for c in range(DC2):
                                nc.tensor.matmul(
                                    ps,
                                    wo_sb[:, 2 * c : 2 * c + 2, m * P : (m + 1) * P],
                                    ctxT_sb[:, 2 * c : 2 * c + 2, q0 : q0 + HQ],
                                    start=(c == 0),
                                    stop=(c == DC2 - 1),
                                    perf_mode=DR,
                                )
                            nc.vector.scalar_tensor_tensor(
                                yT_sb[:, m, q0 : q0 + HQ], ps, RWS, xr, MUL, ADD
                            )
                            acc = (
                                acc8_sb[:, m, q0 : q0 + HQ]
                                if m < D8
                                else acc16_sb[:, m - D8, q0 : q0 + HQ]
                            )
                            nc.scalar.activation(
                                acc, yT_sb[:, m, q0 : q0 + HQ], AF.Copy
                            )
                        yield o_unit
                    # h, ffn-chunk-split by FFN2 dtype
                    h8 = htp.tile([P, F8, HQ], fp8, tag="h8")
                    h16 = htp.tile([P, FC - F8, HQ], bf16, tag="h16")
                    # z staging: FFN1 PSUM lands here via DVE (table-free)
                    # with the b1 bias folded in; gelu then runs batched over
                    # GB chunks in ONE instruction — avoiding the 1.3us ACT
                    # table reload that every EXP<->GELU switch costs. Two
                    # buffers so the next batch's readouts don't wait on the
                    # previous batch's gelu read.
                    GB = 8
                    zbox = []
                    # FFN layer 1: one unit per ffn chunk m (6 matmuls); the
                    # batch-closing unit also emits the batched gelu.
                    for m in range(FC):
                        def f1_unit(m=m, h8=h8, h16=h16, zbox=zbox):
                            if m % GB == 0:
                                z8 = htp.tile([P, GB, HQ], bf16, tag="z8", bufs=2)
                                zbox[:] = [z8]
                            z8 = zbox[0]
                            w1t8 = w1s.tile([P, D8, P], fp8, tag="w1c8")
                            nc.sync.dma_start(w1t8[:], w1a_d[m])
                            w1t16 = w1s.tile([P, DC - D8, P], bf16, tag="w1c16")
                            nc.sync.dma_start(w1t16[:], w1b_d[m])
                            ps = psA.tile([P, HQ], f32, tag="ps")
                            for c in range(F1C):
                                nc.tensor.matmul(
                                    ps,
                                    w1t8[:, 2 * c : 2 * c + 2, :],
                                    acc8_sb[:, 2 * c : 2 * c + 2, q0 : q0 + HQ],
                                    start=(c == 0),
                                    stop=False,
                                    perf_mode=DR,
                                )
                            for k in range(DC - D8):
                                nc.tensor.matmul(
                                    ps,
                                    w1t16[:, k, :],
                                    acc16_sb[:, k, q0 : q0 + HQ],
                                    start=False,
                                    stop=(k == DC - D8 - 1),
                                )
                            nc.vector.tensor_scalar(
                                z8[:, m % GB, :],
                                ps,
                                RWS,
                                b1_sb[:, m : m + 1],
                                MUL,
                                ADD,
                            )
                            if m % GB == GB - 1:
                                m0 = m - GB + 1
                                hout = (
                                    h8[:, m0 : m0 + GB, :]
                                    if m + 1 <= F8
                                    else h16[:, m0 - F8 : m0 - F8 + GB, :]
                                )
                                nc.scalar.activation(hout, z8[:, :, :], AF.Gelu)
                        yield f1_unit
                    # FFN layer 2 + residual: two units per output chunk m;
                    # readout fused on DVE (ACT stays on exp)
                    for m in range(DC):
                        box = []
                        def f2a(m=m, h8=h8, h16=h16, box=box):
                            w2t8 = w2s.tile([P, F8, P], fp8, tag="w2c8")
                            nc.sync.dma_start(w2t8[:], w2a_d[m])
                            w2t16 = w2s.tile([P, FC - F8, P], bf16, tag="w2c16")
                            nc.sync.dma_start(w2t16[:], w2b_d[m])
                            ps = psA.tile([P, HQ], f32, tag="ps")
                            box[:] = [w2t16, ps]
                            for c in range(F2C):
                                nc.tensor.matmul(
                                    ps,
                                    w2t8[:, 2 * c : 2 * c + 2, :],
                                    h8[:, 2 * c : 2 * c + 2, :],
                                    start=(c == 0),
                                    stop=False,
                                    perf_mode=DR,
                                )
                            for k in range(NB1):
                                nc.tensor.matmul(
                                    ps,
                                    w2t16[:, k, :],
                                    h16[:, k, :],
                                    start=False,
                                    stop=False,
                                )
                        def f2b(m=m, h16=h16, box=box):
                            w2t16, ps = box
                            for k in range(NB1, FC - F8):
                                nc.tensor.matmul(
                                    ps,
                                    w2t16[:, k, :],
                                    h16[:, k, :],
                                    start=False,
                                    stop=(k == FC - F8 - 1),
                                )
                            ot = outp.tile([P, HQ], f32, tag="ot")
                            nc.vector.tensor_scalar(
                                ot, ps, RWS, b2_sb[:, m : m + 1], MUL, ADD
                            )
                            nc.vector.tensor_add(
                                ot, ot, yT_sb[:, m, q0 : q0 + HQ]
                            )
                            nc.sync.dma_start(
                                outT_d[:, m, q0 : q0 + HQ], ot
                            )
                        yield f2a
                        yield f2b

                # Phase B: half-1 attention with half-0 O+FFN interleaved.
                units = list(ffn_units(half=0))
                ui = [0]

                def drain(n):
                    for _ in range(n):
                        if ui[0] < len(units):
                            units[ui[0]]()
                            ui[0] += 1

                for p in range(NPAIR):
                    def hook(skh):
                        drain(1)
                    attn_pair(p, half=1, spread_hook=hook)
                drain(len(units))  # leftovers

                # Phase C: half-1 O + FFN, PE-dense
                for u in ffn_units(half=1):
                    u()

    nc.compile()
    return nc


def _get_program():
    if "nc" not in _CACHE:
        _CACHE["nc"] = _build_program()
    return _CACHE["nc"]


def _wlayout(W):
    # [D_in, D_out] -> [P, D_in//P, D_out]
    return np.ascontiguousarray(
        W.reshape(W.shape[0] // P, P, W.shape[1]).transpose(1, 0, 2)
    )


def _blayout(b):
    # [D] -> [P, D//P]
    return np.ascontiguousarray(b.reshape(b.shape[0] // P, P).T)


def prepare_in_maps(x, Wq, bq, Wk, bk, Wv, bv, Wo, bo, W1, b1, W2, b2):
    x = np.asarray(x, np.float32)
    Wq = np.asarray(Wq, np.float32)
    bq = np.asarray(bq, np.float32)
    Wk = np.asarray(Wk, np.float32)
    bk = np.asarray(bk, np.float32)
    Wv = np.asarray(Wv, np.float32)
    bv = np.asarray(bv, np.float32)
    Wo = np.asarray(Wo, np.float32)
    bo = np.asarray(bo, np.float32)
    W1 = np.asarray(W1, np.float32)
    b1 = np.asarray(b1, np.float32)
    W2 = np.asarray(W2, np.float32)
    b2 = np.asarray(b2, np.float32)

    scale = DH ** -0.5
    shared = {
        # x8 under the generic x16: wq also carries the 1/sqrt(DH) q scale
        "wq": _wlayout(Wq * (scale * WS * 8.0)).astype(FP8),
        "wk": _wlayout(Wk * WS).astype(FP8),
        "wv": _wlayout(Wv * WS).astype(FP8),
        "wo": _wlayout(Wo * (WS / CTXS)).astype(FP8),
        "w1a": np.ascontiguousarray(
            (W1 * WS).reshape(DC, P, FC, P).transpose(2, 1, 0, 3)[:, :, :D8, :]
        ).astype(FP8),
        "w1b": np.ascontiguousarray(
            (W1 * WS).reshape(DC, P, FC, P).transpose(2, 1, 0, 3)[:, :, D8:, :]
        ).astype(BF16),
        "w2a": np.ascontiguousarray(
            (W2 * WS).reshape(FC, P, DC, P).transpose(2, 1, 0, 3)[:, :, :F8, :]
        ).astype(FP8),
        "w2b": np.ascontiguousarray(
            (W2 * WS).reshape(FC, P, DC, P).transpose(2, 1, 0, 3)[:, :, F8:, :]
        ).astype(BF16),
        "bq": _blayout(bq * scale),
        "bk": _blayout(bk),
        "bvb": np.ascontiguousarray(np.broadcast_to(bv, (P, D))).astype(BF16),
        "b1": _blayout(b1),
        "b2": _blayout(b2),
    }

    in_maps = []
    for c in range(NCORES):
        b_idx, half = divmod(c, 2)
        xb = x[b_idx]  # [S, D]
        xbT = xb.T  # [D, S]
        xT = np.ascontiguousarray(
            xbT.reshape(DC, P, S).transpose(1, 0, 2)
        ).astype(FP8)
        xqT = np.ascontiguousarray(
            xbT[:, half * SQ : (half + 1) * SQ]
            .reshape(DC, P, SQ)
            .transpose(1, 0, 2)
        ).astype(FP8)
        xres = np.ascontiguousarray(
            (xbT[:, half * SQ : (half + 1) * SQ] + bo[:, None])
            .reshape(DC, P, SQ)
            .transpose(1, 0, 2)
        ).astype(np.float32)
        in_maps.append(dict(shared, xT=xT, xqT=xqT, xres=xres))
    return in_maps


def assemble_out(results):
    out = np.empty((B, S, D), np.float32)
    for c in range(NCORES):
        b_idx, half = divmod(c, 2)
        outT = results[c]["outT"]  # [P, DC, SQ]
        out[b_idx, half * SQ : (half + 1) * SQ] = (
            outT.transpose(1, 0, 2).reshape(D, SQ).T
        )
    return out


def kernel(**inputs):
    from concourse.bass_utils import run_bass_kernel_spmd

    in_maps = prepare_in_maps(**inputs)
    nc = _get_program()
    res = run_bass_kernel_spmd(nc, in_maps, core_ids=list(range(NCORES)))
    return assemble_out(res.results)


# revision 28
# speedup vs baseline: 1.0135x; 1.0135x over previous
"""Trainium2 Bass kernel for a dense transformer layer (attention + FFN).

Sharding: 8 shards = (batch b, sequence half) pairs. Each core computes the
full K/V projections for its batch (2x redundant) and Q/attention/FFN for its
1024-token query slice. No cross-core communication.

On-device layout is feature-major (transposed): activations live as
[feature, token] so every matmul is lhsT.T @ rhs with natural weight layouts.

Precision: QKV/O projections and the attention ctx matmuls run fp8e4 with
DoubleRow perf mode (256-row contraction per instruction, ~1.9x PE speedup);
kt/qt/E/v/ctx live in fp8. The FFN stays bf16 end to end — fp8 there costs
~2.4e-2 relative error (measured), over the 2e-2 gate. Weights are pre-scaled
x16 on the host to sit in fp8e4's normal range; the 1/16 is folded into the
PSUM->SBUF readout ops. ctx is scaled x32 via the softmax reciprocal
(compensated in Wo). fp32 PSUM accumulation; residual stream in fp32.

Schedule: the query slice is split in two 512-token halves. Phase A runs
K/Q/V projections and half-0 attention (ACT-bound on exp). Phase B runs
half-1 attention on ACT while the PE stream interleaves half-0's O
projection and FFN between attention matmuls — keeping the PE array dense
(avoids HAM down-throttle) and overlapping the exp floor with FFN compute.
Phase C finishes half-1's O projection and FFN.
"""

import numpy as np
import ml_dtypes

B, S, D = 4, 2048, 1024
H, DH, F = 16, 64, 4096
P = 128
NCORES = 8
SQ = B * S // NCORES  # 1024 query tokens per core
HQ = SQ // 2  # 512-token query half
DC = D // P  # 8 feature chunks
DC2 = DC // 2  # 4 double chunks (DoubleRow)
FC = F // P  # 32 ffn chunks
SKC = S // P  # 16 key chunks
SKC2 = SKC // 2
NPAIR = H // 2  # 8 head pairs (2 heads per 128-feature chunk)

WS = 16.0  # host-side fp8 weight scale
RWS = 1.0 / WS
CTXS = 32.0  # ctx fp8 scale (folded into softmax recip; compensated in Wo)

# Partial-fp8 FFN: the first F1C (of DC2=4) 256-row double-chunks of the
# FFN1 contraction and the first F2C (of FC2=16) of FFN2 run fp8 DoubleRow;
# the rest stay bf16. (2, 8) measured 1.92e-2 on HW — too close to the 2e-2
# gate; (2, 4) sims at ~1.55e-2.
F1C = 2
F2C = 4
D8 = 2 * F1C  # y/W1 chunks stored fp8
F8 = 2 * F2C  # h/W2 chunks stored fp8
# f2a/f2b matmul split point so the two units are balanced
NB1 = max(0, (F2C + FC - F8) // 2 - F2C)

BF16 = ml_dtypes.bfloat16
FP8 = ml_dtypes.float8_e4m3

_CACHE = {}


def _build_program():
    import concourse.mybir as mybir
    import concourse.tile as tile
    from concourse import bacc

    f32 = mybir.dt.float32
    bf16 = mybir.dt.bfloat16
    fp8 = mybir.dt.float8e4
    AF = mybir.ActivationFunctionType
    DR = mybir.MatmulPerfMode.DoubleRow
    MUL = mybir.AluOpType.mult
    ADD = mybir.AluOpType.add

    nc = bacc.Bacc("TRN2", target_bir_lowering=False, debug=False, num_devices=NCORES)

    xT_d = nc.dram_tensor("xT", [P, DC, S], fp8, kind="ExternalInput")
    xqT_d = nc.dram_tensor("xqT", [P, DC, SQ], fp8, kind="ExternalInput")
    xres_d = nc.dram_tensor("xres", [P, DC, SQ], f32, kind="ExternalInput")
    wq_d = nc.dram_tensor("wq", [P, DC, D], fp8, kind="ExternalInput")
    wk_d = nc.dram_tensor("wk", [P, DC, D], fp8, kind="ExternalInput")
    wv_d = nc.dram_tensor("wv", [P, DC, D], fp8, kind="ExternalInput")
    wo_d = nc.dram_tensor("wo", [P, DC, D], fp8, kind="ExternalInput")
    w1a_d = nc.dram_tensor("w1a", [FC, P, D8, P], fp8, kind="ExternalInput")
    w1b_d = nc.dram_tensor("w1b", [FC, P, DC - D8, P], bf16, kind="ExternalInput")
    w2a_d = nc.dram_tensor("w2a", [DC, P, F8, P], fp8, kind="ExternalInput")
    w2b_d = nc.dram_tensor("w2b", [DC, P, FC - F8, P], bf16, kind="ExternalInput")
    bq_d = nc.dram_tensor("bq", [P, DC], f32, kind="ExternalInput")
    bk_d = nc.dram_tensor("bk", [P, DC], f32, kind="ExternalInput")
    bvb_d = nc.dram_tensor("bvb", [P, D], bf16, kind="ExternalInput")
    b1_d = nc.dram_tensor("b1", [P, FC], f32, kind="ExternalInput")
    b2_d = nc.dram_tensor("b2", [P, DC], f32, kind="ExternalInput")
    outT_d = nc.dram_tensor("outT", [P, DC, SQ], f32, kind="ExternalOutput")

    with tile.TileContext(nc) as tc:
        with (
            tc.tile_pool(name="psA", bufs=2, space="PSUM") as psA,
            tc.tile_pool(name="psS", bufs=2, space="PSUM") as psS,
            tc.tile_pool(name="psC", bufs=2, space="PSUM") as psC,
            tc.tile_pool(name="biasp", bufs=1) as biasp,
            tc.tile_pool(name="ctxp", bufs=1) as ctxp,
            tc.tile_pool(name="ep", bufs=6) as ep,
            tc.tile_pool(name="rp", bufs=2) as rp,
            tc.tile_pool(name="rbp", bufs=2) as rbp,
        ):
            bq_sb = biasp.tile([P, DC], f32)
            bk_sb = biasp.tile([P, DC], f32)
            b1_sb = biasp.tile([P, FC], f32)
            b2_sb = biasp.tile([P, DC], f32)
            nc.scalar.dma_start(bq_sb[:], bq_d[:])
            nc.scalar.dma_start(bk_sb[:], bk_d[:])
            nc.scalar.dma_start(b1_sb[:], b1_d[:])
            nc.scalar.dma_start(b2_sb[:], b2_d[:])

            ctxT_sb = ctxp.tile([P, DC, SQ], fp8)
            # y (FFN input activation), feature-chunk-split by FFN1 dtype
            acc8_sb = ctxp.tile([P, D8, SQ], fp8)
            acc16_sb = ctxp.tile([P, DC - D8, SQ], bf16)
            wo_sb = ctxp.tile([P, DC, D], fp8)
            v_sb = ctxp.tile([P, SKC, H, DH + 1], fp8)
            kt_all = ctxp.tile([P, NPAIR, S], fp8)
            qt_all = ctxp.tile([P, NPAIR, SQ], fp8)
            yT_sb = ctxp.tile([P, DC, SQ], f32)

            def attn_pair(p, half, spread_hook=None):
                """Attention for head pair (2p, 2p+1), queries
                [half*HQ, half*HQ+HQ). spread_hook(skh) emits filler PE work.

                Software-pipelined: scores run one sk-chunk-pair ahead of ctx
                so the in-order PE stream never serializes the next scores
                behind exp — ACT stays continuously busy on exp."""
                q0 = half * HQ
                pc0 = psC.tile([P, HQ], f32, tag="pc")
                pc1 = psC.tile([P, HQ], f32, tag="pc")
                E2s = [None] * SKC2

                def emit_scores(skh):
                    # E2: exp(scores), laid [key, chunk-parity, headA|headB]
                    # = the ctx DoubleRow moving operand.
                    E2 = ep.tile([P, 2, 2 * HQ], fp8)
                    E2s[skh] = E2
                    for hs in range(2):
                        sk = 2 * skh + hs
                        ss = psS.tile([P, 2 * HQ], f32)
                        nc.tensor.matmul(
                            ss[:, 0:HQ],
                            kt_all[0:64, p, sk * P : (sk + 1) * P],
                            qt_all[0:64, p, q0 : q0 + HQ],
                            start=True,
                            stop=True,
                        )
                        nc.tensor.matmul(
                            ss[:, HQ : 2 * HQ],
                            kt_all[64:128, p, sk * P : (sk + 1) * P],
                            qt_all[64:128, p, q0 : q0 + HQ],
                            start=True,
                            stop=True,
                        )
                        nc.scalar.activation(E2[:, hs, :], ss, AF.Exp)

                emit_scores(0)
                for skh in range(SKC2):
                    if skh + 1 < SKC2:
                        emit_scores(skh + 1)
                    # filler PE work lands between next-scores and this ctx so
                    # the PE covers the exp latency instead of stalling on E2
                    if spread_hook is not None:
                        spread_hook(skh)
                    E2 = E2s[skh]
                    nc.tensor.matmul(
                        pc0[:65],
                        v_sb[:, 2 * skh : 2 * skh + 2, 2 * p, :],
                        E2[:, :, 0:HQ],
                        start=(skh == 0),
                        stop=(skh == SKC2 - 1),
                        perf_mode=DR,
                    )
                    nc.tensor.matmul(
                        pc1[:65],
                        v_sb[:, 2 * skh : 2 * skh + 2, 2 * p + 1, :],
                        E2[:, :, HQ : 2 * HQ],
                        start=(skh == 0),
                        stop=(skh == SKC2 - 1),
                        perf_mode=DR,
                    )
                # softmax normalization: ctx * (CTXS / rowsum); the CTXS fp8
                # range scale is divided back out in Wo. (approx recip is ~18
                # correct bits, plenty for a softmax denom)
                for hh, pc in ((0, pc0), (1, pc1)):
                    s0 = rp.tile([1, HQ], f32, tag="s")
                    nc.vector.tensor_scalar_mul(s0, pc[64:65, :], 1.0 / CTXS)
                    r0 = rp.tile([1, HQ], f32, tag="r")
                    nc.vector.reciprocal_approx_fast(r0, s0)
                    rb0 = rbp.tile([64, HQ], f32, tag="rb")
                    nc.gpsimd.partition_broadcast(rb0, r0)
                    nc.vector.tensor_mul(
                        ctxT_sb[64 * hh : 64 * hh + 64, p, q0 : q0 + HQ],
                        pc[0:64, :],
                        rb0,
                    )

            # ---------------- Phase A: projections + half-0 attention -------
            with (
                tc.tile_pool(name="abp", bufs=1) as abp,
                tc.tile_pool(name="wvp", bufs=1) as wvp,
                tc.tile_pool(name="ws", bufs=3) as ws,
            ):
                # x^T in 4 column-chunk tiles so V/K matmuls start after the
                # first chunk lands rather than after the full DMA.
                xTs = [
                    abp.tile([P, DC, 512], fp8, tag=f"xT{c}", name=f"xT{c}")
                    for c in range(4)
                ]
                wvs = [
                    wvp.tile([P, DC, 512], fp8, tag=f"wv{c}", name=f"wv{c}")
                    for c in range(2)
                ]
                bvb_sb = abp.tile([P, D], bf16)
                xqT_sb = abp.tile([P, DC, SQ], fp8)
                # startup DMA priority comes from in-queue FIFO order: the
                # first V matmul's inputs (x and wv chunks 0-1) lead their
                # queues; bulk transfers follow behind them
                nc.sync.dma_start(xTs[0][:, 0:2, :], xT_d[:, 0:2, 0:512])
                nc.gpsimd.dma_start(wvs[0][:, 0:2, :], wv_d[:, 0:2, 0:512])
                nc.sync.dma_start(xTs[0][:, 2:DC, :], xT_d[:, 2:DC, 0:512])
                nc.gpsimd.dma_start(wvs[0][:, 2:DC, :], wv_d[:, 2:DC, 0:512])
                nc.sync.dma_start(xTs[1][:], xT_d[:, :, 512:1024])
                nc.sync.dma_start(xTs[2][:], xT_d[:, :, 1024:1536])
                nc.sync.dma_start(xTs[3][:], xT_d[:, :, 1536:2048])
                nc.gpsimd.dma_start(wvs[1][:], wv_d[:, :, 512:1024])
                nc.gpsimd.dma_start(xqT_sb[:], xqT_d[:])
                nc.scalar.dma_start(wo_sb[:], wo_d[:])
                nc.sync.dma_start(bvb_sb[:], bvb_d[:])

                # V projection, token-major: v[sk, dv] (+ ones column per
                # head). fp8: it is the ctx DoubleRow stationary operand.
                nc.vector.memset(v_sb[:, :, :, DH : DH + 1], 1.0)

                def emit_v(nv, sks, h0=0, h1=8):
                    nh = h1 - h0
                    for sk in sks:
                        xt = xTs[sk // 4]
                        co = (sk % 4) * P
                        ps = psA.tile([P, 512], f32, tag="ps")
                        for c in range(DC2):
                            nc.tensor.matmul(
                                ps[:, : nh * DH],
                                xt[:, 2 * c : 2 * c + 2, co : co + P],
                                wvs[nv][:, 2 * c : 2 * c + 2, h0 * DH : h1 * DH],
                                start=(c == 0),
                                stop=(c == DC2 - 1),
                                perf_mode=DR,
                            )
                        nc.vector.scalar_tensor_tensor(
                            v_sb[:, sk, nv * 8 + h0 : nv * 8 + h1, 0:DH],
                            ps[:, : nh * DH].rearrange("p (h d) -> p h d", h=nh),
                            RWS,
                            bvb_sb[
                                :, nv * 512 + h0 * DH : nv * 512 + h1 * DH
                            ].rearrange("p (h d) -> p h d", h=nh),
                            MUL,
                            ADD,
                        )

                def kq_units(p):
                    """K/Q projection PE work for pair p as 6 ~1-1.5us units.
                    Weight DMAs are issued at queue-build time (prefetch)."""
                    wkt = ws.tile([P, DC, P], fp8, tag="wchunk")
                    nc.sync.dma_start(wkt[:], wk_d[:, :, p * P : (p + 1) * P])
                    wqt = ws.tile([P, DC, P], fp8, tag="wchunk")
                    nc.sync.dma_start(wqt[:], wq_d[:, :, p * P : (p + 1) * P])
                    units = []
                    for n in range(S // 512):
                        def ku(n=n):
                            ps = psA.tile([P, 512], f32, tag="ps")
                            for c in range(DC2):
                                nc.tensor.matmul(
                                    ps,
                                    wkt[:, 2 * c : 2 * c + 2, :],
                                    xTs[n][:, 2 * c : 2 * c + 2, :],
                                    start=(c == 0),
                                    stop=(c == DC2 - 1),
                                    perf_mode=DR,
                                )
                            nc.vector.tensor_scalar(
                                kt_all[:, p, n * 512 : (n + 1) * 512],
                                ps,
                                RWS,
                                bk_sb[:, p : p + 1],
                                MUL,
                                ADD,
                            )
                        units.append(ku)
                    # wq is x16 overall on the host (x128 on Wq*scale for fp8
                    # range); divide the full 128 back out in the readout.
                    for n in range(SQ // 512):
                        def qu(n=n):
                            ps = psA.tile([P, 512], f32, tag="ps")
                            for c in range(DC2):
                                nc.tensor.matmul(
                                    ps,
                                    wqt[:, 2 * c : 2 * c + 2, :],
                                    xqT_sb[:, 2 * c : 2 * c + 2, n * 512 : (n + 1) * 512],
                                    start=(c == 0),
                                    stop=(c == DC2 - 1),
                                    perf_mode=DR,
                                )
                            nc.vector.tensor_scalar(
                                qt_all[:, p, n * 512 : (n + 1) * 512],
                                ps,
                                RWS / 8.0,
                                bq_sb[:, p : p + 1],
                                MUL,
                                ADD,
                            )
                        units.append(qu)
                    return units

                # V(nv=1) chunk counts per pair (heads 8-15, needed from
                # pair 4 on — must complete by end of pair 3)
                V1_PLAN = {0: 2, 1: 5, 2: 5, 3: 4}

                emit_v(0, range(SKC))
                for u in kq_units(0):
                    u()
                v1_next = 0
                for p in range(NPAIR):
                    queue = []
                    if p + 1 < NPAIR:
                        queue.extend(kq_units(p + 1))
                    for _ in range(V1_PLAN.get(p, 0)):
                        queue.append(lambda sk=v1_next: emit_v(1, [sk]))
                        v1_next += 1

                    def hook(skh, queue=queue):
                        # drain >=1 unit/slot, catching up so the queue
                        # empties by the last slot of the pair
                        rem_slots = SKC2 - skh
                        n = max(1, -(-len(queue) // rem_slots))
                        for _ in range(min(n, len(queue))):
                            queue.pop(0)()

                    attn_pair(p, half=0, spread_hook=hook)
                    for u in queue:
                        u()

            # ------- Phases B+C: half-1 attention overlapped with half-0
            # O-projection + FFN, then half-1 O + FFN ---------------------
            with (
                tc.tile_pool(name="htp", bufs=1) as htp,
                tc.tile_pool(name="w1s", bufs=3) as w1s,
                tc.tile_pool(name="w2s", bufs=2) as w2s,
                tc.tile_pool(name="xrp", bufs=3) as xrp,
                tc.tile_pool(name="outp", bufs=4) as outp,
            ):
                def ffn_units(half):
                    """Yield per-unit closures of O-proj + FFN PE work for one
                    query half. Each unit is ~1-2us of PE work."""
                    q0 = half * HQ
                    # O projection + residual: one unit per feature chunk m
                    for m in range(DC):
                        def o_unit(m=m):
                            xr = xrp.tile([P, HQ], f32, tag="xr")
                            nc.sync.dma_start(xr[:], xres_d[:, m, q0 : q0 + HQ])
                            ps = psA.tile([P, HQ], f32)
                            for c in range(DC2):
                                nc.tensor.matmul(
                                    ps,
                                    wo_sb[:, 2 * c : 2 * c + 2, m * P : (m + 1) * P],
                                    ctxT_sb[:, 2 * c : 2 * c + 2, q0 : q0 + HQ],
                                    start=(c == 0),
                                    stop=(c == DC2 - 1),
                                    perf_mode=DR,
                                )
                            nc.vector.scalar_tensor_tensor(
                                yT_sb[:, m, q0 : q0 + HQ], ps, RWS, xr, MUL, ADD
                            )
                            acc = (
                                acc8_sb[:, m, q0 : q0 + HQ]
                                if m < D8
                                else acc16_sb[:, m - D8, q0 : q0 + HQ]
                            )
                            nc.scalar.activation(
                                acc, yT_sb[:, m, q0 : q0 + HQ], AF.Copy
                            )
                        yield o_unit
                    # h, ffn-chunk-split by FFN2 dtype
                    h8 = htp.tile([P, F8, HQ], fp8, tag="h8")
                    h16 = htp.tile([P, FC - F8, HQ], bf16, tag="h16")
                    # z staging: FFN1 PSUM lands here via DVE (table-free)
                    # with the b1 bias folded in; gelu then runs batched over
                    # GB chunks in ONE instruction — avoiding the 1.3us ACT
                    # table reload that every EXP<->GELU switch costs. Two
                    # buffers so the next batch's readouts don't wait on the
                    # previous batch's gelu read.
                    GB = 8
                    zbox = []
                    # FFN layer 1: one unit per ffn chunk m (6 matmuls); the
                    # batch-closing unit also emits the batched gelu.
                    for m in range(FC):
                        def f1_unit(m=m, h8=h8, h16=h16, zbox=zbox):
                            if m % GB == 0:
                                z8 = htp.tile([P, GB, HQ], bf16, tag="z8", bufs=2)
                                zbox[:] = [z8]
                            z8 = zbox[0]
                            w1t8 = w1s.tile([P, D8, P], fp8, tag="w1c8")
                            nc.sync.dma_start(w1t8[:], w1a_d[m])
                            w1t16 = w1s.tile([P, DC - D8, P], bf16, tag="w1c16")
                            nc.sync.dma_start(w1t16[:], w1b_d[m])
                            ps = psA.tile([P, HQ], f32, tag="ps")
                            for c in range(F1C):
                                nc.tensor.matmul(
                                    ps,
                                    w1t8[:, 2 * c : 2 * c + 2, :],
                                    acc8_sb[:, 2 * c : 2 * c + 2, q0 : q0 + HQ],
                                    start=(c == 0),
                                    stop=False,
                                    perf_mode=DR,
                                )
                            for k in range(DC - D8):
                                nc.tensor.matmul(
                                    ps,
                                    w1t16[:, k, :],
                                    acc16_sb[:, k, q0 : q0 + HQ],
                                    start=False,
                                    stop=(k == DC - D8 - 1),
                                )
                            nc.vector.tensor_scalar(
                                z8[:, m % GB, :],
                                ps,
                                RWS,
                                b1_sb[:, m : m + 1],
                                MUL,
                                ADD,
                            )
                            if m % GB == GB - 1:
                                m0 = m - GB + 1
                                hout = (
                                    h8[:, m0 : m0 + GB, :]
                                    if m + 1 <= F8
                                    else h16[:, m0 - F8 : m0 - F8 + GB, :]
                                )
                                nc.scalar.activation(hout, z8[:, :, :], AF.Gelu)
                        yield f1_unit
                    # FFN layer 2 + residual: two units per output chunk m;
                    # readout fused on DVE (ACT stays on exp)
                    for m in range(DC):
                        box = []
                        def f2a(m=m, h8=h8, h16=h16, box=box):
                            w2t8 = w2s.tile([P, F8, P], fp8, tag="w2c8")
                            nc.sync.dma_start(w2t8[:], w2a_d[m])
                            w2t16 = w2s.tile([P, FC - F8, P], bf16, tag="w2c16")
                            nc.sync.dma_start(w2t16[:], w2b_d[m])
                            ps = psA.tile([P, HQ], f32, tag="ps")
                            box[:] = [w2t16, ps]
                            for c in range(F2C):
                                nc.tensor.matmul(
                                    ps,
                                    w2t8[:, 2 * c : 2 * c + 2, :],
                                    h8[:, 2 * c : 2 * c + 2, :],
                                    start=(c == 0),
                                    stop=False,
                                    perf_mode=DR,
                                )
                            for k in range(NB1):
                                nc.tensor.matmul(
                                    ps,
                                    w2t16[:, k, :],
                                    h16[:, k, :],
                                    start=False,
                                    stop=False,
                                )
                        def f2b(m=m, h16=h16, box=box):
                            w2t16, ps = box
                            for k in range(NB1, FC - F8):
                                nc.tensor.matmul(
                                    ps,
                                    w2t16[:, k, :],
                                    h16[:, k, :],
                                    start=False,
                                    stop=(k == FC - F8 - 1),
                                )
                            ot = outp.tile([P, HQ], f32, tag="ot")
                            nc.vector.tensor_scalar(
                                ot, ps, RWS, b2_sb[:, m : m + 1], MUL, ADD
                            )
                            nc.vector.tensor_add(
                                ot, ot, yT_sb[:, m, q0 : q0 + HQ]
                            )
                            nc.sync.dma_start(
                                outT_d[:, m, q0 : q0 + HQ], ot
                            )
                        yield f2a
                        yield f2b

                # Phase B: half-1 attention with half-0 O+FFN interleaved.
                units = list(ffn_units(half=0))
                ui = [0]

                def drain(n):
                    for _ in range(n):
                        if ui[0] < len(units):
                            units[ui[0]]()
                            ui[0] += 1

                for p in range(NPAIR):
                    def hook(skh):
                        drain(1)
                    attn_pair(p, half=1, spread_hook=hook)
                drain(len(units))  # leftovers

                # Phase C: half-1 O + FFN, PE-dense
                for u in ffn_units(half=1):
                    u()

    nc.compile()
    return nc


def _get_program():
    if "nc" not in _CACHE:
        _CACHE["nc"] = _build_program()
    return _CACHE["nc"]


def _wlayout(W):
    # [D_in, D_out] -> [P, D_in//P, D_out]
    return np.ascontiguousarray(
        W.reshape(W.shape[0] // P, P, W.shape[1]).transpose(1, 0, 2)
    )


def _blayout(b):
    # [D] -> [P, D//P]
    return np.ascontiguousarray(b.reshape(b.shape[0] // P, P).T)


def prepare_in_maps(x, Wq, bq, Wk, bk, Wv, bv, Wo, bo, W1, b1, W2, b2):
    x = np.asarray(x, np.float32)
    Wq = np.asarray(Wq, np.float32)
    bq = np.asarray(bq, np.float32)
    Wk = np.asarray(Wk, np.float32)
    bk = np.asarray(bk, np.float32)
    Wv = np.asarray(Wv, np.float32)
    bv = np.asarray(bv, np.float32)
    Wo = np.asarray(Wo, np.float32)
    bo = np.asarray(bo, np.float32)
    W1 = np.asarray(W1, np.float32)
    b1 = np.asarray(b1, np.float32)
    W2 = np.asarray(W2, np.float32)
    b2 = np.asarray(b2, np.float32)

    scale = DH ** -0.5
    shared = {
        # x8 under the generic x16: wq also carries the 1/sqrt(DH) q scale
        "wq": _wlayout(Wq * (scale * WS * 8.0)).astype(FP8),
        "wk": _wlayout(Wk * WS).astype(FP8),
        "wv": _wlayout(Wv * WS).astype(FP8),
        "wo": _wlayout(Wo * (WS / CTXS)).astype(FP8),
        "w1a": np.ascontiguousarray(
            (W1 * WS).reshape(DC, P, FC, P).transpose(2, 1, 0, 3)[:, :, :D8, :]
        ).astype(FP8),
        "w1b": np.ascontiguousarray(
            (W1 * WS).reshape(DC, P, FC, P).transpose(2, 1, 0, 3)[:, :, D8:, :]
        ).astype(BF16),
        "w2a": np.ascontiguousarray(
            (W2 * WS).reshape(FC, P, DC, P).transpose(2, 1, 0, 3)[:, :, :F8, :]
        ).astype(FP8),
        "w2b": np.ascontiguousarray(
            (W2 * WS).reshape(FC, P, DC, P).transpose(2, 1, 0, 3)[:, :, F8:, :]
        ).astype(BF16),
        "bq": _blayout(bq * scale),
        "bk": _blayout(bk),
        "bvb": np.ascontiguousarray(np.broadcast_to(bv, (P, D))).astype(BF16),
        "b1": _blayout(b1),
        "b2": _blayout(b2),
    }

    in_maps = []
    for c in range(NCORES):
        b_idx, half = divmod(c, 2)
        xb = x[b_idx]  # [S, D]
        xbT = xb.T  # [D, S]
        xT = np.ascontiguousarray(
            xbT.reshape(DC, P, S).transpose(1, 0, 2)
        ).astype(FP8)
        xqT = np.ascontiguousarray(
            xbT[:, half * SQ : (half + 1) * SQ]
            .reshape(DC, P, SQ)
            .transpose(1, 0, 2)
        ).astype(FP8)
        xres = np.ascontiguousarray(
            (xbT[:, half * SQ : (half + 1) * SQ] + bo[:, None])
            .reshape(DC, P, SQ)
            .transpose(1, 0, 2)
        ).astype(np.float32)
        in_maps.append(dict(shared, xT=xT, xqT=xqT, xres=xres))
    return in_maps


def assemble_out(results):
    out = np.empty((B, S, D), np.float32)
    for c in range(NCORES):
        b_idx, half = divmod(c, 2)
        outT = results[c]["outT"]  # [P, DC, SQ]
        out[b_idx, half * SQ : (half + 1) * SQ] = (
            outT.transpose(1, 0, 2).reshape(D, SQ).T
        )
    return out


def kernel(**inputs):
    from concourse.bass_utils import run_bass_kernel_spmd

    in_maps = prepare_in_maps(**inputs)
    nc = _get_program()
    res = run_bass_kernel_spmd(nc, in_maps, core_ids=list(range(NCORES)))
    return assemble_out(res.results)
